# revision 7
# baseline (speedup 1.0000x reference)
"""Trainium2 Bass kernel for nn_MultiHeadAttention_56504589746463.

Math (per batch b, x = queries[b], all derived from the reference):
  q = x @ Wq.T, k = x @ Wk.T, v = x @ Wv.T       (per-head split, DH=64)
  scores_h = q_h @ k_h.T / 8, masked over k >= valid_len
  attn = softmax(scores)  (no row-max needed: |scores| is small)
  out = (attn @ v) @ Wo.T ; pooled = mean_s out ; logits -> log_softmax

Key reduction: the final output only needs pooled = mean over s, so per head
we only need  U_h[d] = sum_q (1/Z_q) * G_h[d, q]  with
  G_h[d, q]  = sum_k v_h[k, d] * E_h[k, q],   E = exp(scores/8),
  Z_q        = sum_k E_h[k, q].
Z is obtained for free by appending a ones-column to v (so the G matmul
computes [v | 1]^T @ E -> [65, S], row 64 = Z).

Masking: the host zeroes x rows with index >= valid_len. Then k columns for
invalid positions are exactly 0 -> scores 0 -> E = 1, and v rows are 0 so G is
unpolluted; the host subtracts (S - valid_len) from Z to remove the spurious
ones. The final 1/Z weighting, mean-pooling, Wo/Wc projections and log_softmax
are O(B*D^2) and run on the host.

Sharding: data-parallel over batch, 1 batch per NeuronCore (8 cores).
"""

import numpy as np
import ml_dtypes

B, S, D, H, DH = 8, 2048, 512, 8, 64
NCORES = 8
VCOL = 65  # per-head columns in the augmented V: 64 v-dims + 1 ones column

_NC_CACHE = {}


def _build_nc(s=S):
    import concourse.bass as bass
    import concourse.tile as tile
    import concourse.mybir as mybir
    from concourse import bacc

    f32 = mybir.dt.float32
    bf16 = mybir.dt.bfloat16
    nkt = s // 128        # number of 128-row k tiles
    ndc = D // 128        # 4 contraction chunks of the model dim

    nc = bacc.Bacc("TRN2", target_bir_lowering=False, debug=False,
                   num_devices=NCORES)
    xT = nc.dram_tensor("xT", [D, s], bf16, kind="ExternalInput").ap()
    wqT = nc.dram_tensor("wqT", [D, D], bf16, kind="ExternalInput").ap()
    wkT = nc.dram_tensor("wkT", [D, D], bf16, kind="ExternalInput").ap()
    wvA = nc.dram_tensor("wvA", [D, H * VCOL], bf16, kind="ExternalInput").ap()
    g = nc.dram_tensor("g", [H, VCOL, s], f32, kind="ExternalOutput").ap()

    # q-column chunks of the scores free dim (<=1024 so scores psum is 2 banks)
    qchunks = []
    off = 0
    while off < s:
        sz = min(1024, s - off)
        qchunks.append((off, sz))
        off += sz

    def emit(tc):
        from contextlib import ExitStack
        with ExitStack() as ctx:
            const = ctx.enter_context(tc.tile_pool(name="const", bufs=1))

            xT_sb = [const.tile([128, s], bf16, name=f"xT{i}", tag=f"xT{i}") for i in range(ndc)]
            wq_sb = [const.tile([128, D], bf16, name=f"wq{i}", tag=f"wq{i}") for i in range(ndc)]
            wk_sb = [const.tile([128, D], bf16, name=f"wk{i}", tag=f"wk{i}") for i in range(ndc)]
            wv_sb = [const.tile([128, H * VCOL], bf16, name=f"wv{i}", tag=f"wv{i}") for i in range(ndc)]
            for i in range(ndc):
                nc.sync.dma_start(out=xT_sb[i], in_=xT[i * 128:(i + 1) * 128, :])
                nc.sync.dma_start(out=wq_sb[i], in_=wqT[i * 128:(i + 1) * 128, :])
                nc.sync.dma_start(out=wk_sb[i], in_=wkT[i * 128:(i + 1) * 128, :])
                nc.sync.dma_start(out=wv_sb[i], in_=wvA[i * 128:(i + 1) * 128, :])

            qT_sb = [const.tile([128, s], bf16, name=f"qT{i}", tag=f"qT{i}") for i in range(ndc)]
            kT_sb = [const.tile([128, s], bf16, name=f"kT{i}", tag=f"kT{i}") for i in range(ndc)]
            vaug_sb = [const.tile([128, H * VCOL], bf16, name=f"va{i}", tag=f"va{i}")
                       for i in range(nkt)]

            # ---- Phase 1: projections ----
            with tc.tile_pool(name="pps", bufs=6, space="PSUM") as pps:
                pchunks = [(c0, min(512, s - c0)) for c0 in range(0, s, 512)]
                for dst_sb, w_sb in ((qT_sb, wq_sb), (kT_sb, wk_sb)):
                    for t in range(ndc):
                        pss = [pps.tile([128, 512], f32, name="proj", tag="proj", bufs=6)
                               for _ in pchunks]
                        for dc in range(ndc):
                            lhsT = w_sb[dc][:, t * 128:(t + 1) * 128]
                            for c, (c0, cw) in enumerate(pchunks):
                                nc.tensor.matmul(
                                    pss[c][:, :cw],
                                    lhsT,
                                    xT_sb[dc][:, c0:c0 + cw],
                                    start=(dc == 0), stop=(dc == ndc - 1))
                        for c, (c0, cw) in enumerate(pchunks):
                            nc.vector.tensor_copy(
                                dst_sb[t][:, c0:c0 + cw], pss[c][:, :cw])
                # v (augmented): out [128 s-rows, 520]; N split 260+260
                half = (H * VCOL) // 2
                for st in range(nkt):
                    for hh in range(2):
                        ps = pps.tile([128, half], f32, name="vproj", tag="vproj", bufs=2)
                        for dc in range(ndc):
                            nc.tensor.matmul(
                                ps,
                                xT_sb[dc][:, st * 128:(st + 1) * 128],
                                wv_sb[dc][:, hh * half:(hh + 1) * half],
                                start=(dc == 0), stop=(dc == ndc - 1))
                        nc.vector.tensor_copy(
                            vaug_sb[st][:, hh * half:(hh + 1) * half], ps)
                    ones_view = vaug_sb[st].rearrange(
                        "p (h c) -> p h c", c=VCOL)[:, :, 64:65]
                    nc.vector.memset(ones_view, 1.0)

            # ---- Phase 2: attention ----
            with tc.tile_pool(name="scps", bufs=2, space="PSUM") as scps, \
                 tc.tile_pool(name="gps", bufs=1, space="PSUM") as gps, \
                 tc.tile_pool(name="epool", bufs=3) as epool, \
                 tc.tile_pool(name="gout", bufs=2) as gout:
                for h in range(H):
                    tq = h // 2
                    po = (h % 2) * 64
                    g_ps = gps.tile([VCOL, s], f32, name="gtile", tag="gtile")
                    for kt in range(nkt):
                        ktile = kT_sb[tq][po:po + 64, kt * 128:(kt + 1) * 128]
                        vslice = vaug_sb[kt][:, h * VCOL:(h + 1) * VCOL]
                        for (qo, qn) in qchunks:
                            sc = scps.tile([128, 1024], f32, name="sc", tag="sc")
                            for nn in range(0, qn, 512):
                                nw = min(512, qn - nn)
                                nc.tensor.matmul(
                                    sc[:, nn:nn + nw],
                                    ktile,
                                    qT_sb[tq][po:po + 64, qo + nn:qo + nn + nw],
                                    start=True, stop=True)
                            import concourse.mybir as _mb
                            e_sb = epool.tile([128, 1024], bf16, name="e", tag="e")
                            nc.scalar.activation(
                                e_sb[:, :qn], sc[:, :qn],
                                _mb.ActivationFunctionType.Exp,
                                scale=0.125)
                            for nn in range(0, qn, 512):
                                nw = min(512, qn - nn)
                                nc.tensor.matmul(
                                    g_ps[:, qo + nn:qo + nn + nw],
                                    vslice,
                                    e_sb[:, nn:nn + nw],
                                    start=(kt == 0), stop=(kt == nkt - 1))
                    g_sb = gout.tile([VCOL, s], f32, name="gsb", tag="gsb")
                    nc.vector.tensor_copy(g_sb, g_ps)
                    nc.sync.dma_start(out=g[h], in_=g_sb)

    with tile.TileContext(nc) as tc:
        emit(tc)
    nc.compile()
    return nc


def get_nc(s=S):
    if s not in _NC_CACHE:
        _NC_CACHE[s] = _build_nc(s)
    return _NC_CACHE[s]


def host_prepare(queries, valid_lens, Wq, Wk, Wv, s=S):
    """Build per-core input maps."""
    bf = ml_dtypes.bfloat16
    vl = np.asarray(valid_lens).astype(np.int64)
    wqT = np.ascontiguousarray(np.asarray(Wq, dtype=np.float32).T).astype(bf)
    wkT = np.ascontiguousarray(np.asarray(Wk, dtype=np.float32).T).astype(bf)
    WvT = np.asarray(Wv, dtype=np.float32).T  # [D, D]
    wvA = np.zeros((D, H * VCOL), dtype=np.float32)
    for h in range(H):
        wvA[:, h * VCOL:h * VCOL + DH] = WvT[:, h * DH:(h + 1) * DH]
    wvA = wvA.astype(bf)
    q_np = np.asarray(queries, dtype=np.float32)
    in_maps = []
    for b in range(B):
        x = q_np[b].copy()
        x[int(vl[b]):, :] = 0.0
        xTa = np.ascontiguousarray(x.T).astype(bf)
        in_maps.append({"xT": xTa, "wqT": wqT, "wkT": wkT, "wvA": wvA})
    return in_maps, vl


def host_finish(g_results, vl, Wo, Wc, bc, s=S):
    """g_results: list of B arrays [H, VCOL, s] f32."""
    Wo64 = np.asarray(Wo, dtype=np.float64)
    Wc64 = np.asarray(Wc, dtype=np.float64)
    bc64 = np.asarray(bc, dtype=np.float64)
    nb = len(g_results)
    out = np.zeros((nb, 2), dtype=np.float32)
    for b in range(nb):
        gf = np.asarray(g_results[b], dtype=np.float64)  # [H, 65, s]
        Gv = gf[:, :DH, :]                               # [H, 64, s]
        Z = gf[:, DH, :] - (s - int(vl[b]))              # [H, s]
        r = 1.0 / Z
        U = np.einsum('hdq,hq->hd', Gv, r)               # [H, 64]
        pooled_attn = U.reshape(D) / s
        pooled = pooled_attn @ Wo64.T
        logits = pooled @ Wc64.T + bc64
        m = logits.max()
        ls = logits - m - np.log(np.exp(logits - m).sum())
        out[b] = ls.astype(np.float32)
    return out


def kernel(queries, keys, values, valid_lens, Wq, Wk, Wv, Wo, Wc, bc):
    from concourse.bass_utils import run_bass_kernel_spmd
    nc = get_nc()
    in_maps, vl = host_prepare(queries, valid_lens, Wq, Wk, Wv)
    res = run_bass_kernel_spmd(nc, in_maps, core_ids=list(range(NCORES)))
    g_results = [res.results[b]["g"] for b in range(B)]
    return host_finish(g_results, vl, Wo, Wc, bc)


# revision 10
# speedup vs baseline: 2.8338x; 2.8338x over previous
"""Trainium2 Bass kernel for nn_MultiHeadAttention_56504589746463.

Math (per batch b, x = queries[b], all derived from the reference):
  q = x @ Wq.T, k = x @ Wk.T, v = x @ Wv.T       (per-head split, DH=64)
  scores_h = q_h @ k_h.T / 8, masked over k >= valid_len
  attn = softmax(scores)  (no row-max needed: |scores| is small)
  out = (attn @ v) @ Wo.T ; pooled = mean_s out ; logits -> log_softmax

Key reduction: the final output only needs pooled = mean over s, so per head
we only need  U_h[d] = sum_q (1/Z_q) * G_h[d, q]  with
  G_h[d, q]  = sum_k v_h[k, d] * E_h[k, q],   E = exp(scores/8),
  Z_q        = sum_k E_h[k, q].
Z is obtained for free by appending a ones-column to v (so the G matmul
computes [v | 1]^T @ E -> [65, S], row 64 = Z).

Masking: the host zeroes x rows with index >= valid_len. Then k columns for
invalid positions are exactly 0 -> scores 0 -> E = 1, and v rows are 0 so G is
unpolluted; the host subtracts (S - valid_len) from Z to remove the spurious
ones. The final 1/Z weighting, mean-pooling, Wo/Wc projections and log_softmax
are O(B*D^2) and run on the host.

Sharding: data-parallel over batch, 1 batch per NeuronCore (8 cores).
"""

import numpy as np
import ml_dtypes

B, S, D, H, DH = 8, 2048, 512, 8, 64
NCORES = 8
VCOL = 65  # per-head columns in the augmented V: 64 v-dims + 1 ones column

_NC_CACHE = {}


def _build_nc(s=S, repeats=1):
    import concourse.bass as bass
    import concourse.tile as tile
    import concourse.mybir as mybir
    from concourse import bacc

    f32 = mybir.dt.float32
    bf16 = mybir.dt.bfloat16
    nkt = s // 128        # number of 128-row k tiles
    ndc = D // 128        # 4 contraction chunks of the model dim

    nc = bacc.Bacc("TRN2", target_bir_lowering=False, debug=False,
                   num_devices=NCORES)
    xT = nc.dram_tensor("xT", [D, s], bf16, kind="ExternalInput").ap()
    wqT = nc.dram_tensor("wqT", [D, D], bf16, kind="ExternalInput").ap()
    wkT = nc.dram_tensor("wkT", [D, D], bf16, kind="ExternalInput").ap()
    wvA = nc.dram_tensor("wvA", [D, H * VCOL], bf16, kind="ExternalInput").ap()
    g = nc.dram_tensor("g", [H, VCOL, s], f32, kind="ExternalOutput").ap()

    # q-column chunks of the scores free dim (<=1024 so scores psum is 2 banks)
    qchunks = []
    off = 0
    while off < s:
        sz = min(1024, s - off)
        qchunks.append((off, sz))
        off += sz

    def emit(tc):
        from contextlib import ExitStack
        with ExitStack() as ctx:
            const = ctx.enter_context(tc.tile_pool(name="const", bufs=1))

            xT_sb = [const.tile([128, s], bf16, name=f"xT{i}", tag=f"xT{i}") for i in range(ndc)]
            wq_sb = [const.tile([128, D], bf16, name=f"wq{i}", tag=f"wq{i}") for i in range(ndc)]
            wk_sb = [const.tile([128, D], bf16, name=f"wk{i}", tag=f"wk{i}") for i in range(ndc)]
            wv_sb = [const.tile([128, H * VCOL], bf16, name=f"wv{i}", tag=f"wv{i}") for i in range(ndc)]
            for i in range(ndc):
                nc.sync.dma_start(out=xT_sb[i], in_=xT[i * 128:(i + 1) * 128, :])
                nc.sync.dma_start(out=wq_sb[i], in_=wqT[i * 128:(i + 1) * 128, :])
                nc.sync.dma_start(out=wk_sb[i], in_=wkT[i * 128:(i + 1) * 128, :])
                nc.sync.dma_start(out=wv_sb[i], in_=wvA[i * 128:(i + 1) * 128, :])

            def one_pass():
                qT_sb = [const.tile([128, s], bf16, name=f"qT{i}", tag=f"qT{i}")
                         for i in range(ndc)]
                kT_sb = [const.tile([128, s], bf16, name=f"kT{i}", tag=f"kT{i}")
                         for i in range(ndc)]
                vaug_sb = [const.tile([128, H * VCOL], bf16, name=f"va{i}", tag=f"va{i}")
                           for i in range(nkt)]

                # ---- Phase 1: projections ----
                with tc.tile_pool(name="pps", bufs=6, space="PSUM") as pps:
                    pchunks = [(c0, min(512, s - c0)) for c0 in range(0, s, 512)]
                    for dst_sb, w_sb in ((qT_sb, wq_sb), (kT_sb, wk_sb)):
                        for t in range(ndc):
                            pss = [pps.tile([128, 512], f32, name="proj", tag="proj", bufs=6)
                                   for _ in pchunks]
                            for dc in range(ndc):
                                lhsT = w_sb[dc][:, t * 128:(t + 1) * 128]
                                for c, (c0, cw) in enumerate(pchunks):
                                    nc.tensor.matmul(
                                        pss[c][:, :cw],
                                        lhsT,
                                        xT_sb[dc][:, c0:c0 + cw],
                                        start=(dc == 0), stop=(dc == ndc - 1))
                            for c, (c0, cw) in enumerate(pchunks):
                                nc.vector.tensor_copy(
                                    dst_sb[t][:, c0:c0 + cw], pss[c][:, :cw])
                    # v (augmented): out [128 s-rows, 520]; N split 260+260
                    half = (H * VCOL) // 2
                    for st in range(nkt):
                        for hh in range(2):
                            ps = pps.tile([128, half], f32, name="vproj", tag="vproj", bufs=2)
                            for dc in range(ndc):
                                nc.tensor.matmul(
                                    ps,
                                    xT_sb[dc][:, st * 128:(st + 1) * 128],
                                    wv_sb[dc][:, hh * half:(hh + 1) * half],
                                    start=(dc == 0), stop=(dc == ndc - 1))
                            nc.vector.tensor_copy(
                                vaug_sb[st][:, hh * half:(hh + 1) * half], ps)
                        ones_view = vaug_sb[st].rearrange(
                            "p (h c) -> p h c", c=VCOL)[:, :, 64:65]
                        nc.vector.memset(ones_view, 1.0)

                # ---- Phase 2: attention ----
                import concourse.mybir as _mb
                with tc.tile_pool(name="scps", bufs=2, space="PSUM") as scps, \
                     tc.tile_pool(name="gps", bufs=1, space="PSUM") as gps, \
                     tc.tile_pool(name="epool", bufs=3) as epool, \
                     tc.tile_pool(name="gout", bufs=2) as gout:
                    for h in range(H):
                        tq = h // 2
                        po = (h % 2) * 64
                        g_ps = gps.tile([VCOL, s], f32, name="gtile", tag="gtile")
                        for kt in range(nkt):
                            ktile = kT_sb[tq][po:po + 64, kt * 128:(kt + 1) * 128]
                            vslice = vaug_sb[kt][:, h * VCOL:(h + 1) * VCOL]
                            for (qo, qn) in qchunks:
                                sc = scps.tile([128, 1024], f32, name="sc", tag="sc")
                                for nn in range(0, qn, 512):
                                    nw = min(512, qn - nn)
                                    nc.tensor.matmul(
                                        sc[:, nn:nn + nw],
                                        ktile,
                                        qT_sb[tq][po:po + 64, qo + nn:qo + nn + nw],
                                        start=True, stop=True)
                                e_sb = epool.tile([128, 1024], bf16, name="e", tag="e")
                                nc.scalar.activation(
                                    e_sb[:, :qn], sc[:, :qn],
                                    _mb.ActivationFunctionType.Exp,
                                    scale=0.125)
                                for nn in range(0, qn, 512):
                                    nw = min(512, qn - nn)
                                    nc.tensor.matmul(
                                        g_ps[:, qo + nn:qo + nn + nw],
                                        vslice,
                                        e_sb[:, nn:nn + nw],
                                        start=(kt == 0), stop=(kt == nkt - 1))
                        g_sb = gout.tile([VCOL, s], f32, name="gsb", tag="gsb")
                        nc.vector.tensor_copy(g_sb, g_ps)
                        nc.sync.dma_start(out=g[h], in_=g_sb)

            for _rep in range(repeats):
                one_pass()

    with tile.TileContext(nc) as tc:
        emit(tc)
    nc.compile()
    return nc


def get_nc(s=S):
    if s not in _NC_CACHE:
        _NC_CACHE[s] = _build_nc(s)
    return _NC_CACHE[s]


def host_prepare(queries, valid_lens, Wq, Wk, Wv, s=S):
    """Build per-core input maps."""
    bf = ml_dtypes.bfloat16
    vl = np.asarray(valid_lens).astype(np.int64)
    wqT = np.ascontiguousarray(np.asarray(Wq, dtype=np.float32).T).astype(bf)
    wkT = np.ascontiguousarray(np.asarray(Wk, dtype=np.float32).T).astype(bf)
    WvT = np.asarray(Wv, dtype=np.float32).T  # [D, D]
    wvA = np.zeros((D, H * VCOL), dtype=np.float32)
    for h in range(H):
        wvA[:, h * VCOL:h * VCOL + DH] = WvT[:, h * DH:(h + 1) * DH]
    wvA = wvA.astype(bf)
    q_np = np.asarray(queries, dtype=np.float32)
    in_maps = []
    for b in range(B):
        x = q_np[b].copy()
        x[int(vl[b]):, :] = 0.0
        xTa = np.ascontiguousarray(x.T).astype(bf)
        in_maps.append({"xT": xTa, "wqT": wqT, "wkT": wkT, "wvA": wvA})
    return in_maps, vl


def host_finish(g_results, vl, Wo, Wc, bc, s=S):
    """g_results: list of B arrays [H, VCOL, s] f32."""
    Wo64 = np.asarray(Wo, dtype=np.float64)
    Wc64 = np.asarray(Wc, dtype=np.float64)
    bc64 = np.asarray(bc, dtype=np.float64)
    nb = len(g_results)
    out = np.zeros((nb, 2), dtype=np.float32)
    for b in range(nb):
        gf = np.asarray(g_results[b], dtype=np.float64)  # [H, 65, s]
        Gv = gf[:, :DH, :]                               # [H, 64, s]
        Z = gf[:, DH, :] - (s - int(vl[b]))              # [H, s]
        r = 1.0 / Z
        U = np.einsum('hdq,hq->hd', Gv, r)               # [H, 64]
        pooled_attn = U.reshape(D) / s
        pooled = pooled_attn @ Wo64.T
        logits = pooled @ Wc64.T + bc64
        m = logits.max()
        ls = logits - m - np.log(np.exp(logits - m).sum())
        out[b] = ls.astype(np.float32)
    return out


def kernel(queries, keys, values, valid_lens, Wq, Wk, Wv, Wo, Wc, bc):
    from concourse.bass_utils import run_bass_kernel_spmd
    nc = get_nc()
    in_maps, vl = host_prepare(queries, valid_lens, Wq, Wk, Wv)
    res = run_bass_kernel_spmd(nc, in_maps, core_ids=list(range(NCORES)))
    g_results = [res.results[b]["g"] for b in range(B)]
    return host_finish(g_results, vl, Wo, Wc, bc)


# revision 11
# speedup vs baseline: 4.0129x; 1.4161x over previous
"""V2: q-sharded, valid-length-specialized attention kernel.

Each core processes q rows [c*256, (c+1)*256) of EVERY batch, over only the
valid k range of each batch (padded to 512). Per (batch, head, q-tile):
  scores[q, k] = qT_h^T kT_h   (layout A: q on partitions, k free)
  E = exp(scores/8)  on ACT, with accum_out giving Z_q (row sums) for free
  r = 1 / (Z - n_pad)          (host-zeroed invalid x rows make E=1 there)
  W8_h[k] += sum_q r_q E[q,k]  (M=1 matmul, 4 heads col-tiled into one psum)
Host: W8 is additive over cores; U_h = W8_h[:valid] @ x[:valid]; then the
tiny Wv/Wo/Wc projections, pooling and log_softmax in float64.

All loop bounds specialize on valid_lens at build time (cached per tuple).
"""

import numpy as np
import ml_dtypes

B, S, D, H, DH = 8, 2048, 512, 8, 64
NCORES = 8
QSL = 256           # q rows per core
NQT = 2             # q tiles of 128

_NC_CACHE = {}


PAD_GRAN = 2

def _pads_from_valids(valids, gran=None):
    g = PAD_GRAN if gran is None else gran
    return tuple(int(min(S, ((int(v) + g - 1) // g) * g)) for v in valids)


def build_v2(valids, nb=B, repeats=1, gran=None, inter=True, pack=True, zeng='vector'):
    """valids: tuple of per-batch valid lengths (compile-time constants)."""
    import concourse.tile as tile
    import concourse.mybir as mybir
    from concourse import bacc

    f32 = mybir.dt.float32
    bf16 = mybir.dt.bfloat16
    Exp = mybir.ActivationFunctionType.Exp
    pads = _pads_from_valids(valids, gran)
    KP = sum(pads)
    koffs = np.cumsum([0] + list(pads))[:-1]
    padmax = max(pads)
    ndc = D // 128

    nc = bacc.Bacc("TRN2", target_bir_lowering=False, debug=False,
                   num_devices=NCORES)
    # xq: per-core q slices, batch-pair packed: panel p = [x_{2p} | x_{2p+1}]
    xq = nc.dram_tensor("xq", [nb // 2, D, 2 * QSL], bf16,
                        kind="ExternalInput").ap()
    xk = nc.dram_tensor("xk", [D, KP], bf16, kind="ExternalInput").ap()
    wqT = nc.dram_tensor("wqT", [D, D], bf16, kind="ExternalInput").ap()
    wkT = nc.dram_tensor("wkT", [D, D], bf16, kind="ExternalInput").ap()
    # w8o[b, hg, j, :] = W8 for head hg*4+j (partition 32*j), this core's q rows
    w8o = nc.dram_tensor("w8o", [nb, NQT, 2, 4, padmax], bf16,
                         kind="ExternalOutput").ap()

    def kchunks(pad):
        out = []
        off = 0
        while off < pad:
            w = min(1024, pad - off)
            out.append((off, w))
            off += w
        return out

    def emit(tc):
        from contextlib import ExitStack
        with ExitStack() as ctx:
            const = ctx.enter_context(tc.tile_pool(name="const", bufs=1))
            wq_sb = [const.tile([128, D], bf16, name=f"wq{i}", tag=f"wq{i}")
                     for i in range(ndc)]
            wk_sb = [const.tile([128, D], bf16, name=f"wk{i}", tag=f"wk{i}")
                     for i in range(ndc)]
            xq_sb = [[const.tile([128, 2 * QSL], bf16, name=f"xq{p}_{i}",
                                 tag=f"xq{p}_{i}") for i in range(ndc)]
                     for p in range(nb // 2)]
            for i in range(ndc):
                nc.sync.dma_start(out=wq_sb[i], in_=wqT[i * 128:(i + 1) * 128, :])
                nc.sync.dma_start(out=wk_sb[i], in_=wkT[i * 128:(i + 1) * 128, :])
            for p in range(nb // 2):
                for i in range(ndc):
                    nc.sync.dma_start(out=xq_sb[p][i],
                                      in_=xq[p, i * 128:(i + 1) * 128, :])
            # qT2[p][t]: [128 dout, 2*QSL] bf16 (pair-packed q projections)
            qT2 = [[const.tile([128, 2 * QSL], bf16, name=f"qT{p}_{t}",
                               tag=f"qT{p}_{t}") for t in range(ndc)]
                   for p in range(nb // 2)]

            work = ctx.enter_context(tc.tile_pool(name="work", bufs=2))
            scps = ctx.enter_context(tc.tile_pool(name="scps", bufs=2,
                                                  space="PSUM"))
            w8ps_pool = ctx.enter_context(tc.tile_pool(name="w8ps", bufs=1,
                                                       space="PSUM"))
            epool = ctx.enter_context(tc.tile_pool(name="epool", bufs=10))
            zpool = ctx.enter_context(tc.tile_pool(name="zpool", bufs=16))
            rb_ring = [const.tile([128, 32], bf16, name=f"rb{i}", tag=f"rb{i}")
                       for i in range(12)]
            for rtile in rb_ring:
                nc.vector.memset(rtile, 0.0)
            rb_ctr = [0]

            def one_pass(rep):
                # ---- qT projections (pair-packed, N=2*QSL<=512) ----
                for p in range(nb // 2):
                    for t in range(ndc):
                        ps = scps.tile([128, 1024], f32, name="sc", tag="sc")
                        for dc in range(ndc):
                            nc.tensor.matmul(
                                ps[:, :2 * QSL],
                                wq_sb[dc][:, t * 128:(t + 1) * 128],
                                xq_sb[p][dc],
                                start=(dc == 0), stop=(dc == ndc - 1))
                        nc.vector.tensor_copy(qT2[p][t], ps[:, :2 * QSL])

                # ---- per batch: kT projection + attention ----
                # kT projection for batch b is emitted as "units" that are
                # interleaved into batch b-1's attention groups so the PE
                # work hides under ACT's exp stream instead of stalling it.
                def start_kproj(b):
                    pad = pads[b]
                    koff = int(koffs[b])
                    xk_sb = [work.tile([128, padmax], bf16, name=f"xk{dc}",
                                       tag=f"xk{dc}") for dc in range(ndc)]
                    for dc in range(ndc):
                        nc.sync.dma_start(
                            out=xk_sb[dc][:, :pad],
                            in_=xk[dc * 128:(dc + 1) * 128, koff:koff + pad])
                    kT_sb = [work.tile([128, padmax], bf16, name=f"kT{t}",
                                       tag=f"kT{t}") for t in range(ndc)]

                    def unit(tc0):
                        t, c0 = tc0
                        cw = min(1024, pad - c0)
                        ps = scps.tile([128, 1024], f32, name="sc", tag="sc")
                        for dc in range(ndc):
                            for nn in range(0, cw, 512):
                                nw = min(512, cw - nn)
                                nc.tensor.matmul(
                                    ps[:, nn:nn + nw],
                                    wk_sb[dc][:, t * 128:(t + 1) * 128],
                                    xk_sb[dc][:, c0 + nn:c0 + nn + nw],
                                    start=(dc == 0), stop=(dc == ndc - 1))
                        nc.vector.tensor_copy(kT_sb[t][:, c0:c0 + cw],
                                              ps[:, :cw])

                    units = [(t, c0) for t in range(ndc)
                             for c0 in range(0, pad, 1024)]
                    return kT_sb, unit, list(units)

                n_groups = 4 if inter else 0
                kproj = {0: start_kproj(0)}
                for u in kproj[0][2]:
                    kproj[0][1](u)
                kproj[0] = (kproj[0][0], None, [])

                for b in range(nb):
                    pad = pads[b]
                    koff = int(koffs[b])
                    n_inv = float(pad - int(valids[b]))
                    kT_sb = kproj[b][0]
                    if b + 1 < nb:
                        kproj[b + 1] = start_kproj(b + 1)
                        nxt_unit, nxt_units = kproj[b + 1][1], kproj[b + 1][2]
                        if not inter:
                            for u in nxt_units:
                                nxt_unit(u)
                            nxt_units = []
                    else:
                        nxt_unit, nxt_units = None, []
                    per_group = (len(nxt_units) + 3) // 4
                    gi = 0

                    for hg in range(2):
                        for qt in range(NQT):
                            w8ps = w8ps_pool.tile([128, padmax], f32,
                                                  name="w8t", tag="w8t")
                            ehs, rbs = [], []
                            zps_by_j = {}
                            # scores+exp for the 4 heads, emitted in parity
                            # pairs so consecutive matmuls use different PE
                            # row groups (rows 0-63 vs 64-127) and overlap.
                            for jp in (0, 2):
                                scs = []
                                for j in (jp, jp + 1):
                                    h = hg * 4 + j
                                    e_h = epool.tile([128, padmax], bf16,
                                                     name="e", tag="e")
                                    ehs.append(e_h)
                                for (c0, cw) in kchunks(pad):
                                    pair_sc = [scps.tile([128, 1024], f32,
                                                         name="sc", tag="sc")
                                               for _ in range(2)]
                                    nns = list(range(0, cw, 512))
                                    if pack:
                                        order = [(nn, jo) for nn in nns
                                                 for jo in range(2)]
                                    else:
                                        order = [(nn, jo) for jo in range(2)
                                                 for nn in nns]
                                    for nn, jo in order:
                                        nw = min(512, cw - nn)
                                        j = jp + jo
                                        h = hg * 4 + j
                                        t = h // 2
                                        po = (h % 2) * 64
                                        qcol = (b % 2) * QSL + qt * 128
                                        lhsq = qT2[b // 2][t][po:po + 64,
                                                              qcol:qcol + 128]
                                        nc.tensor.matmul(
                                            pair_sc[jo][:, nn:nn + nw],
                                            lhsq,
                                            kT_sb[t][po:po + 64,
                                                     c0 + nn:c0 + nn + nw],
                                            start=True, stop=True)
                                    for jo, j in enumerate((jp, jp + 1)):
                                        zp = zpool.tile([128, 1], f32,
                                                        name="zp", tag="zp")
                                        nc.scalar.activation(
                                            ehs[j][:, c0:c0 + cw],
                                            pair_sc[jo][:, :cw], Exp,
                                            scale=0.125, accum_out=zp)
                                        if c0 == 0:
                                            zps_by_j[j] = [zp]
                                        else:
                                            zps_by_j[j].append(zp)
                                zeng_ = nc.gpsimd if zeng == 'pool' else nc.vector
                                for j in (jp, jp + 1):
                                    zps = zps_by_j[j]
                                    zsum = zps[0]
                                    if len(zps) > 1:
                                        zt = zpool.tile([128, 1], f32,
                                                        name="zt", tag="zt")
                                        zeng_.tensor_add(zt, zps[0], zps[1])
                                        zsum = zt
                                    if n_inv != 0.0:
                                        zc = zpool.tile([128, 1], f32,
                                                        name="zc", tag="zc")
                                        zeng_.tensor_scalar_add(
                                            out=zc, in0=zsum, scalar1=-n_inv)
                                        zsum = zc
                                    rb = rb_ring[rb_ctr[0] % len(rb_ring)]
                                    rb_ctr[0] += 1
                                    with nc.allow_low_precision(
                                            reason="r is a bf16 matmul weight"):
                                        nc.vector.reciprocal(rb[:, 0:1], zsum)
                                    rbs.append(rb)
                            # W8: 4 col-tiled matmuls interleaved across heads
                            nns = list(range(0, pad, 512))
                            if pack:
                                worder = [(nn, j) for nn in nns
                                          for j in range(4)]
                            else:
                                worder = [(nn, j) for j in range(4)
                                          for nn in nns]
                            for nn, j in worder:
                                nw = min(512, pad - nn)
                                nc.tensor.matmul(
                                    w8ps[32 * j:32 * j + 32, nn:nn + nw],
                                    rbs[j],
                                    ehs[j][:, nn:nn + nw],
                                    start=True, stop=True,
                                    tile_position=(0, 32 * j))
                            w8sb = work.tile([128, padmax], bf16,
                                             name="w8sb", tag="w8sb", bufs=3)
                            nc.vector.tensor_copy(w8sb[:, :pad], w8ps[:, :pad])
                            w8v = w8sb.rearrange("(j r) k -> j r k", r=32)
                            nc.sync.dma_start(out=w8o[b, qt, hg, :, :pad],
                                              in_=w8v[:, 0, :pad])
                            for u in nxt_units[gi * per_group:
                                               (gi + 1) * per_group]:
                                nxt_unit(u)
                            gi += 1

            for rep in range(repeats):
                one_pass(rep)

    with tile.TileContext(nc) as tc:
        emit(tc)
    nc.compile()
    return nc


def get_nc_v2(valids, repeats=1, gran=None, inter=True, pack=True,
              zeng='vector'):
    key = (tuple(int(v) for v in valids), repeats, gran, inter, pack, zeng)
    if key not in _NC_CACHE:
        _NC_CACHE[key] = build_v2(key[0], repeats=repeats, gran=gran,
                                  inter=inter, pack=pack, zeng=zeng)
    return _NC_CACHE[key]


def host_prepare_v2(queries, valid_lens, Wq, Wk, gran=None):
    bf = ml_dtypes.bfloat16
    vl = np.asarray(valid_lens).astype(np.int64)
    valids = tuple(int(v) for v in vl)
    pads = _pads_from_valids(valids, gran)
    KP = sum(pads)
    q_np = np.asarray(queries, dtype=np.float32)
    nb = q_np.shape[0]
    wqT = np.ascontiguousarray(np.asarray(Wq, np.float32).T).astype(bf)
    wkT = np.ascontiguousarray(np.asarray(Wk, np.float32).T).astype(bf)
    # packed, masked k-side input (same for all cores)
    xk = np.zeros((D, KP), dtype=np.float32)
    off = 0
    for b in range(nb):
        v = valids[b]
        xk[:, off:off + v] = q_np[b, :v, :].T
        off += pads[b]
    xk = xk.astype(bf)
    in_maps = []
    for c in range(NCORES):
        xq = np.zeros((nb // 2, D, 2 * QSL), dtype=np.float32)
        for p in range(nb // 2):
            xq[p, :, :QSL] = q_np[2 * p, c * QSL:(c + 1) * QSL, :].T
            xq[p, :, QSL:] = q_np[2 * p + 1, c * QSL:(c + 1) * QSL, :].T
        in_maps.append({"xq": xq.astype(bf), "xk": xk,
                        "wqT": wqT, "wkT": wkT})
    return in_maps, valids, pads


def host_finish_v2(w8_list, valids, pads, queries, Wv, Wo, Wc, bc):
    """w8_list: per-core [nb, NQT, 2, 4, padmax] arrays (bf16)."""
    q_np = np.asarray(queries, dtype=np.float64)
    Wv64 = np.asarray(Wv, dtype=np.float64)
    Wo64 = np.asarray(Wo, dtype=np.float64)
    Wc64 = np.asarray(Wc, dtype=np.float64)
    bc64 = np.asarray(bc, dtype=np.float64)
    nb = q_np.shape[0]
    s = q_np.shape[1]
    out = np.zeros((nb, 2), dtype=np.float32)
    w8sum = np.sum([np.asarray(w, np.float64) for w in w8_list], axis=0)
    w8sum = w8sum.sum(axis=1)                      # sum the q-tile partials
    for b in range(nb):
        v = int(valids[b])
        W8 = w8sum[b].reshape(H, -1)[:, :v]        # [H, valid]
        U = W8 @ q_np[b, :v, :]                    # [H, D]
        pooled_attn = np.zeros(D)
        for h in range(H):
            pooled_attn[h * DH:(h + 1) * DH] = (
                U[h] @ Wv64[h * DH:(h + 1) * DH, :].T)
        pooled_attn /= s
        pooled = pooled_attn @ Wo64.T
        logits = pooled @ Wc64.T + bc64
        m = logits.max()
        out[b] = (logits - m - np.log(np.exp(logits - m).sum())).astype(
            np.float32)
    return out


def kernel(queries, keys, values, valid_lens, Wq, Wk, Wv, Wo, Wc, bc):
    from concourse.bass_utils import run_bass_kernel_spmd
    in_maps, valids, pads = host_prepare_v2(queries, valid_lens, Wq, Wk)
    nc = get_nc_v2(valids)
    res = run_bass_kernel_spmd(nc, in_maps, core_ids=list(range(NCORES)))
    w8_list = [res.results[c]["w8o"] for c in range(NCORES)]
    return host_finish_v2(w8_list, valids, pads, queries, Wv, Wo, Wc, bc)
